# revision 1
# baseline (speedup 1.0000x reference)
"""Trainium2 Bass kernel for nn_AttentionCircuit (mixture-routed attention).

Sharding (8 cores, SPMD single program):
  - mixing (project+combine) token-sharded: core c -> batch c//4, tokens [(c%4)*512, +512)
  - tiny AllGather of h vectors (h_q/h_k/h_v, [64,512] each) within 4-core batch groups
  - restore + attention + W_O head-sharded: core handles 4 heads (via per-core
    sliced R_qk/R_v/W_O inputs) over all 2048 tokens of its batch
  - host sums the 4 partial W_O outputs per batch
"""
import sys
sys.path.insert(0, "/opt/trn_rl_repo")
import numpy as np
from contextlib import ExitStack

import concourse.bacc as bacc
import concourse.mybir as mybir
from concourse import tile
from concourse.masks import make_identity
from concourse.bass_utils import run_bass_kernel_spmd

B, S, D, R, H, DH, N = 2, 2048, 1024, 64, 16, 64, 32
NR = N * R            # 2048
P = 128
TOK = 512             # tokens per core (mixing shard)
HL = 4                # local heads per core
DL = HL * DH          # 256 local head dims
GROUPS = [[0, 1, 2, 3], [4, 5, 6, 7]]
F32 = mybir.dt.float32
F32R = mybir.dt.float32r
MULT = mybir.AluOpType.mult
ADD = mybir.AluOpType.add
AXX = mybir.AxisListType.X
EXP = mybir.ActivationFunctionType.Exp

_CACHED = {}


def _r(ap):
    return ap.bitcast(F32R)


def build():
    nc = bacc.Bacc(None, target_bir_lowering=False)
    dp = lambda name, shape, out=False: nc.declare_dram_parameter(
        name, list(shape), F32, isOutput=out)

    xT_d = dp("xT", [D, TOK])
    fw_d = [dp(n, [TOK, N]) for n in ("fwq", "fwk", "fwv")]
    rwT_d = [dp(n, [N, S]) for n in ("rwqT", "rwkT", "rwvT")]
    Fqk_d = dp("Fqk", [D, NR])
    Fv_d = dp("Fv", [D, NR])
    Rqk_d = dp("Rqk", [NR, DL])
    Rv_d = dp("Rv", [NR, DL])
    WOT_d = dp("WOTs", [DL, D])
    maskU_d = dp("maskU", [P, P])
    out_d = dp("outp", [S, D], out=True)

    tog = [0]

    def cp(out, in_):
        tog[0] ^= 1
        if tog[0]:
            nc.scalar.copy(out, in_)
        else:
            nc.vector.tensor_copy(out, in_)

    with ExitStack() as ctx:
        tc = ctx.enter_context(tile.TileContext(nc))
        const = ctx.enter_context(tc.tile_pool(name="const", bufs=1))
        ident = const.tile([P, P], F32, name="ident")
        make_identity(nc, ident[:])
        maskU = const.tile([P, P], F32, name="maskU")
        nc.sync.dma_start(out=maskU[:], in_=maskU_d[:])

        dram = ctx.enter_context(tc.tile_pool(name="dram", bufs=1, space="DRAM"))
        hT_stack = dram.tile([3 * 64, TOK], F32, name="hTstack")
        hT_gath = dram.tile([4 * 3 * 64, TOK], F32, name="hTgath")

        hpool = ctx.enter_context(tc.tile_pool(name="hpool", bufs=12))
        hTpool = ctx.enter_context(tc.tile_pool(name="hTpool", bufs=3))
        h_sb = {}     # (tensor, u) -> [P, R]

        # ---------------- Phase A/B: projections + combines ----------------
        with tc.tile_pool(name="xF", bufs=8) as xF, \
             tc.tile_pool(name="fw", bufs=12) as fwp, \
             tc.tile_pool(name="tmp", bufs=4) as tmpp, \
             tc.tile_pool(name="psA", bufs=4, space="PSUM") as psA, \
             tc.tile_pool(name="psH", bufs=2, space="PSUM") as psH:
            xT_sb = []
            for d in range(8):
                t = xF.tile([P, TOK], F32, tag="xT", name="xT")
                nc.sync.dma_start(out=_r(t[:]), in_=_r(xT_d[d * P:(d + 1) * P, :]))
                xT_sb.append(t)
            fw_sb = {}
            for ti in range(3):
                for u in range(4):
                    t = fwp.tile([P, N], F32, tag="fw", name="fw")
                    nc.sync.dma_start(out=t[:], in_=fw_d[ti][u * P:(u + 1) * P, :])
                    fw_sb[(ti, u)] = t

            for phase, (F_d, tensors) in enumerate(
                    [(Fqk_d, (0, 1)), (Fv_d, (2,))]):
                F_sb = {}
                for d in range(8):
                    for ns in range(4):
                        t = xF.tile([P, 512], F32, tag="F", name="F", bufs=32)
                        nc.sync.dma_start(
                            out=_r(t[:]),
                            in_=_r(F_d[d * P:(d + 1) * P, ns * 512:(ns + 1) * 512]))
                        F_sb[(d, ns)] = t
                for u in range(4):
                    tmps = {ti: tmpp.tile([P, NR], F32, tag="tmp", name="tmp") for ti in tensors}
                    for ns in range(4):
                        ps = psA.tile([P, 512], F32, name="psA")
                        for d in range(8):
                            nc.tensor.matmul(
                                ps[:], _r(xT_sb[d][:, u * P:(u + 1) * P]),
                                _r(F_sb[(d, ns)][:]),
                                start=(d == 0), stop=(d == 7))
                        p3 = ps[:].rearrange("p (n r) -> p n r", n=8)
                        for ti in tensors:
                            w3 = fw_sb[(ti, u)][:, ns * 8:(ns + 1) * 8] \
                                .unsqueeze(2).broadcast_to([P, 8, R])
                            tv = tmps[ti][:].rearrange("p (r n) -> p n r", r=R)[
                                :, ns * 8:(ns + 1) * 8, :]
                            nc.vector.tensor_tensor(out=tv, in0=p3, in1=w3, op=MULT)
                    for ti in tensors:
                        h = hpool.tile([P, R], F32, tag="h", name="h")
                        nc.vector.reduce_sum(
                            out=h[:],
                            in_=tmps[ti][:].rearrange("p (r n) -> p r n", r=R),
                            axis=AXX)
                        h_sb[(ti, u)] = h

            # transpose h -> hT [64, TOK] and stage for AllGather
            for ti in range(3):
                hT = hTpool.tile([64, TOK], F32, tag="hT", name="hT")
                for u in range(4):
                    tp = psH.tile([R, P], F32, name="psH")
                    nc.tensor.transpose(tp[:], h_sb[(ti, u)][:], ident[:])
                    cp(hT[:, u * P:(u + 1) * P], tp[:])
                nc.sync.dma_start(out=hT_stack[ti * 64:(ti + 1) * 64, :], in_=hT[:])

        nc.gpsimd.collective_compute(
            "AllGather", mybir.AluOpType.bypass, replica_groups=GROUPS,
            ins=[hT_stack.opt()], outs=[hT_gath.opt()])

        # h2[tensor] [P, S]: rows 0-63 and 64-127 both = gathered hT rows
        h2pool = ctx.enter_context(tc.tile_pool(name="h2", bufs=3))
        h2 = []
        gv = hT_gath[:].rearrange("(q kr) t -> q kr t", q=4)
        for ti in range(3):
            t = h2pool.tile([P, S], F32, name="h2")
            src = gv[:, ti * 64:(ti + 1) * 64, :].rearrange("q r t -> r q t")
            for half in range(2):
                nc.sync.dma_start(
                    out=t[half * 64:(half + 1) * 64, :]
                        .rearrange("p (q t) -> p q t", q=4),
                    in_=src)
            h2.append(t)

        # ---------------- Phase C/D: restores (local heads only) ----------------
        qkv_pool = ctx.enter_context(tc.tile_pool(name="qkv", bufs=2))
        QT_sb = [qkv_pool.tile([P, S], F32, tag="QT", name="QT", bufs=2) for _ in range(2)]
        KT_sb = [qkv_pool.tile([P, S], F32, tag="KT", name="KT", bufs=2) for _ in range(2)]
        V_sb = [qkv_pool.tile([P, DL], F32, tag="V", name="V", bufs=16) for _ in range(16)]

        with tc.tile_pool(name="Rp", bufs=16) as Rp, \
             tc.tile_pool(name="gT", bufs=18) as gTp, \
             tc.tile_pool(name="wrep", bufs=4) as wrp, \
             tc.tile_pool(name="psC", bufs=4, space="PSUM") as psC:
            R_sb = {}
            for k in range(16):
                t = Rp.tile([P, DL], F32, tag="R", name="R")
                nc.sync.dma_start(out=_r(t[:]),
                                  in_=_r(Rqk_d[k * P:(k + 1) * P, :]))
                R_sb[k] = t

            def grow_gT(ti, ch):
                tiles = []
                for k in range(16):
                    wr = wrp.tile([P, 512], F32, tag="wr", name="wr")
                    for half in range(2):
                        nn = 2 * k + half
                        nc.sync.dma_start(
                            out=wr[half * 64:(half + 1) * 64, :],
                            in_=rwT_d[ti][nn:nn + 1, ch * 512:(ch + 1) * 512]
                                .broadcast_to([64, 512]))
                    g = gTp.tile([P, 512], F32, tag="gT", name="gT")
                    nc.vector.tensor_mul(_r(g[:]), h2[ti][:, ch * 512:(ch + 1) * 512],
                                         wr[:])
                    tiles.append(g)
                return tiles

            for ti, outs in ((0, QT_sb), (1, KT_sb)):
                for ch in range(4):
                    gT = grow_gT(ti, ch)
                    for dt2 in range(2):
                        ps = psC.tile([P, 512], F32, name="psC")
                        for k in range(16):
                            nc.tensor.matmul(
                                ps[:], _r(R_sb[k][:, dt2 * P:(dt2 + 1) * P]),
                                _r(gT[k][:]), start=(k == 0), stop=(k == 15))
                        cp(
                            _r(outs[dt2][:, ch * 512:(ch + 1) * 512]), ps[:])
            # V (token-major), reload Rv into same slots
            for k in range(16):
                t = Rp.tile([P, DL], F32, tag="R", name="R")
                nc.sync.dma_start(out=_r(t[:]), in_=_r(Rv_d[k * P:(k + 1) * P, :]))
                R_sb[k] = t
            for ch in range(4):
                gT = grow_gT(2, ch)
                for tt in range(4):
                    ps = psC.tile([P, DL], F32, name="psCv")
                    for k in range(16):
                        nc.tensor.matmul(
                            ps[:], _r(gT[k][:, tt * P:(tt + 1) * P]),
                            _r(R_sb[k][:]), start=(k == 0), stop=(k == 15))
                    cp(_r(V_sb[ch * 4 + tt][:]), ps[:])

        # ---------------- Phase E: attention + W_O ----------------
        wot_pool = ctx.enter_context(tc.tile_pool(name="wot", bufs=2))
        WOT_sb = []
        for pr in range(2):
            t = wot_pool.tile([P, D], F32, name="wot")
            nc.sync.dma_start(out=_r(t[:]), in_=_r(WOT_d[pr * P:(pr + 1) * P, :]))
            WOT_sb.append(t)

        with tc.tile_pool(name="Ssb", bufs=2) as Sp, \
             tc.tile_pool(name="expS", bufs=2) as Ep, \
             tc.tile_pool(name="expT", bufs=4) as Tp, \
             tc.tile_pool(name="attnP", bufs=4) as Ap, \
             tc.tile_pool(name="osb", bufs=4) as Op, \
             tc.tile_pool(name="small", bufs=24) as smp, \
             tc.tile_pool(name="psS", bufs=2, space="PSUM") as psS, \
             tc.tile_pool(name="psT", bufs=2, space="PSUM") as psT, \
             tc.tile_pool(name="psAV", bufs=2, space="PSUM") as psAV, \
             tc.tile_pool(name="psWO", bufs=2, space="PSUM") as psWO:
            for qt in range(16):
                L = (qt + 1) * P
                nb = (L + 511) // 512
                pair = [Ap.tile([P, P], F32, tag="ap", name="ap") for _ in range(2)]
                for i in range(HL):
                    qtile, qoff = QT_sb[i // 2], (i % 2) * 64
                    ktile = KT_sb[i // 2]
                    S_sb = Sp.tile([P, S], F32, tag="S", name="S")
                    mxs = []
                    for kb in range(nb):
                        Ls = min(512, L - kb * 512)
                        ps = psS.tile([P, 512], F32, name="psS")
                        nc.tensor.matmul(
                            ps[:, :Ls],
                            _r(qtile[qoff:qoff + 64, qt * P:(qt + 1) * P]),
                            _r(ktile[qoff:qoff + 64, kb * 512:kb * 512 + Ls]),
                            start=True, stop=True)
                        nc.vector.scalar_tensor_tensor(
                            out=ps[:, Ls - P:Ls], in0=maskU[:], scalar=-1e30,
                            in1=ps[:, Ls - P:Ls], op0=MULT, op1=ADD) \
                            if kb == nb - 1 else None
                        mx = smp.tile([P, 1], F32, tag="mx", name="mx")
                        nc.vector.reduce_max(out=mx[:], in_=ps[:, :Ls], axis=AXX)
                        mxs.append(mx)
                        cp(S_sb[:, kb * 512:kb * 512 + Ls],
                                            ps[:, :Ls])
                    m = mxs[0]
                    for mx in mxs[1:]:
                        m2 = smp.tile([P, 1], F32, tag="mx", name="mx")
                        nc.vector.tensor_max(m2[:], m[:], mx[:])
                        m = m2
                    negm = smp.tile([P, 1], F32, tag="mx", name="mx")
                    nc.vector.tensor_scalar_mul(negm[:], m[:], -0.125)
                    denom = smp.tile([P, 1], F32, tag="mx", name="mx")
                    expS = Ep.tile([P, S], F32, tag="e", name="e")
                    nc.scalar.activation(expS[:, :L], S_sb[:, :L], EXP,
                                         bias=negm[:], scale=0.125,
                                         accum_out=denom[:])
                    recip = smp.tile([P, 1], F32, tag="mx", name="mx")
                    nc.vector.reciprocal(recip[:], denom[:])
                    att = psAV.tile([P, DH], F32, name="psAV")
                    nblk = L // P
                    for tb in range(nblk):
                        tp = psT.tile([P, P], F32, name="psT")
                        nc.tensor.transpose(tp[:], expS[:, tb * P:(tb + 1) * P],
                                            ident[:])
                        eT = Tp.tile([P, P], F32, tag="eT", name="eT")
                        cp(_r(eT[:]), tp[:])
                        nc.tensor.matmul(att[:], _r(eT[:]),
                                         _r(V_sb[tb][:, i * DH:(i + 1) * DH]),
                                         start=(tb == 0), stop=(tb == nblk - 1))
                    nc.vector.tensor_scalar_mul(
                        _r(pair[i // 2][:, (i % 2) * 64:(i % 2) * 64 + 64]),
                        att[:], recip[:])
                pairT = []
                for pr in range(2):
                    tp = psT.tile([P, P], F32, name="psT")
                    nc.tensor.transpose(tp[:], pair[pr][:], ident[:])
                    pT = Ap.tile([P, P], F32, tag="apT", name="apT")
                    cp(_r(pT[:]), tp[:])
                    pairT.append(pT)
                for d2h in range(2):
                    ps = psWO.tile([P, 512], F32, name="psWO")
                    for pr in range(2):
                        nc.tensor.matmul(
                            ps[:], _r(pairT[pr][:]),
                            _r(WOT_sb[pr][:, d2h * 512:(d2h + 1) * 512]),
                            start=(pr == 0), stop=(pr == 1))
                    osb = Op.tile([P, 512], F32, tag="osb", name="osb")
                    cp(osb[:], ps[:])
                    nc.sync.dma_start(
                        out=out_d[qt * P:(qt + 1) * P, d2h * 512:(d2h + 1) * 512],
                        in_=osb[:])
    nc.finalize()
    return nc


def kernel(x, fqk_weights_Q, fqk_weights_K, fv_weights,
           rqk_weights_Q, rqk_weights_K, rv_weights,
           f_qk, f_v, r_qk, r_v, W_O):
    x = np.ascontiguousarray(np.asarray(x, np.float32))
    F_qk = np.ascontiguousarray(
        np.asarray(f_qk, np.float32).transpose(1, 0, 2).reshape(D, NR))
    F_v = np.ascontiguousarray(
        np.asarray(f_v, np.float32).transpose(1, 0, 2).reshape(D, NR))
    R_qk = np.ascontiguousarray(np.asarray(r_qk, np.float32).reshape(NR, D))
    R_v = np.ascontiguousarray(np.asarray(r_v, np.float32).reshape(NR, D))
    W_OT = np.ascontiguousarray(np.asarray(W_O, np.float32).T)
    maskU = np.triu(np.full((P, P), 1.0, np.float32), 1)

    fw = [np.asarray(a, np.float32) for a in
          (fqk_weights_Q, fqk_weights_K, fv_weights)]
    rw = [np.asarray(a, np.float32) for a in
          (rqk_weights_Q, rqk_weights_K, rv_weights)]

    in_maps = []
    for c in range(8):
        b, ch = c // 4, c % 4
        t0 = ch * TOK
        hb = ch * HL  # first global head
        m = {
            "xT": np.ascontiguousarray(x[b, t0:t0 + TOK, :].T),
            "Fqk": F_qk, "Fv": F_v,
            "Rqk": np.ascontiguousarray(R_qk[:, hb * DH:hb * DH + DL]),
            "Rv": np.ascontiguousarray(R_v[:, hb * DH:hb * DH + DL]),
            "WOTs": np.ascontiguousarray(W_OT[hb * DH:hb * DH + DL, :]),
            "maskU": maskU,
        }
        for name, arr in zip(("fwq", "fwk", "fwv"), fw):
            m[name] = np.ascontiguousarray(arr[b, t0:t0 + TOK, :])
        for name, arr in zip(("rwqT", "rwkT", "rwvT"), rw):
            m[name] = np.ascontiguousarray(arr[b].T)
        in_maps.append(m)

    if "nc" not in _CACHED:
        _CACHED["nc"] = build()
    res = run_bass_kernel_spmd(_CACHED["nc"], in_maps, list(range(8)))
    out = np.zeros((B, S, D), np.float32)
    for c in range(8):
        out[c // 4] += res.results[c]["outp"]
    return out


if __name__ == "__main__":
    rng = np.random.RandomState(0)
    d = np.load("/tmp/inputs.npz")
    out = kernel(**{k: d[k] for k in d.files})
    ref = np.load("/tmp/ref_out.npy")
    rel = np.linalg.norm(out - ref) / np.linalg.norm(ref)
    print("rel fro err:", rel)



# revision 7
# speedup vs baseline: 4.8270x; 4.8270x over previous
"""Trainium2 Bass kernel for nn_AttentionCircuit (mixture-routed attention).

Wire-transfer-minimal SPMD design (8 cores, single program). The axon tunnel
(~35 MB/s) dominates wall-clock, so every staged tensor is fp16 and sharded
with no cross-core replication of large data:

  - project: token-sharded (core c: global tokens [512c, 512c+512)); the
    F neuron banks are staged D-sharded (128 rows/core) and AllGathered
    on-chip; h vectors AllGathered (tiny).
  - restore: nr-sharded (core c: neurons [4c, 4c+4) = 256 of 2048 nr rows);
    partial Y^T accumulated in f32 and ReduceScattered over the D axis so
    each core ends with its 128-dim head slice (2 heads) for all tokens.
  - attention + W_O: head-sharded (2 heads x 2 batches per core); W_O
    partials ReduceScattered over tokens on-chip; each core returns a
    [512, 1024] fp16 slice of the final output.

On-chip numerics: fp16 matmuls with f32 PSUM accumulation everywhere except
the attention score matmul, which runs fp32r on f32 Q/K (softmax scores here
reach |s|~1900, so Q/K are never rounded below f32 after the restore).
"""
import sys
sys.path.insert(0, "/opt/trn_rl_repo")
import numpy as np
from contextlib import ExitStack

import concourse.bacc as bacc
import concourse.mybir as mybir
from concourse import tile
from concourse.masks import make_identity, make_upper_triangular
from concourse.bass_utils import run_bass_kernel_spmd

B, S, D, R, H, DH, N = 2, 2048, 1024, 64, 16, 64, 32
NR = N * R            # 2048
T = B * S             # 4096 global tokens
P = 128
TOK = 512             # tokens per core (project shard)
NL = 4                # neurons per core (restore shard)
KRL = NL * R          # 256 local nr rows
DL = 128              # local d slice (2 heads) for attention/W_O
GROUPS = [[0, 1, 2, 3, 4, 5, 6, 7]]
F32 = mybir.dt.float32
F16 = mybir.dt.float16
F32R = mybir.dt.float32r
MULT = mybir.AluOpType.mult
ADD = mybir.AluOpType.add
AXX = mybir.AxisListType.X
EXP = mybir.ActivationFunctionType.Exp

_CACHED = {}


def _r(ap):
    return ap.bitcast(F32R)


def build():
    nc = bacc.Bacc(None, target_bir_lowering=False)

    def dp(name, shape, dt=F16, out=False):
        return nc.declare_dram_parameter(name, list(shape), dt, isOutput=out)

    xT_d = dp("xT", [D, TOK])                     # x[tok slice].T
    Fb_d = dp("Fboth", [2 * P, NR])               # [Fqk d-chunk; Fv d-chunk]
    fw_d = [dp(n, [TOK, N]) for n in ("fwq", "fwk", "fwv")]
    rwT_d = [dp(n, [NL, T]) for n in ("rwqT", "rwkT", "rwvT")]
    Rqk_d = dp("Rqk", [KRL, D])                   # nr-rows slice, all d
    Rv_d = dp("Rv", [KRL, D])
    WOT_d = dp("WOT", [DL, D])                    # W_O.T rows [128c:+128]
    out_d = dp("outp", [TOK, D], out=True)        # final rows [512c:+512)

    tog = [0]

    def cp(out, in_):
        tog[0] ^= 1
        if tog[0]:
            nc.scalar.copy(out, in_)
        else:
            nc.vector.tensor_copy(out, in_)

    with ExitStack() as ctx:
        tc = ctx.enter_context(tile.TileContext(nc))
        const = ctx.enter_context(tc.tile_pool(name="const", bufs=1))
        ident32 = const.tile([P, P], F32, name="id32")
        make_identity(nc, ident32[:])
        ident16 = const.tile([P, P], F16, name="id16")
        make_identity(nc, ident16[:])
        maskU = const.tile([P, P], F32, name="maskU")
        make_upper_triangular(nc, maskU[:], val=1.0, diag=False)

        dram = ctx.enter_context(tc.tile_pool(name="dram", bufs=1, space="DRAM"))
        FG = dram.tile([8 * 2 * P, NR], F16, name="FG")          # gathered F
        hT_stack = dram.tile([3 * R, TOK], F16, name="hTstack")
        hT_gath = dram.tile([8 * 3 * R, TOK], F16, name="hTgath")
        yt_part = [dram.tile([D, T], F32, name=f"ytp{i}") for i in range(3)]
        yt_full = [dram.tile([DL, T], F32, name=f"ytf{i}") for i in range(3)]
        out_part = dram.tile([T, D], F32, name="outpart")
        out_rs = dram.tile([TOK, D], F32, name="outrs")

        # -------- AllGather the D-sharded F banks (1MB in, 8MB out) --------
        # (collectives cannot read IO tensors directly: bounce via SBUF)
        Fstage = dram.tile([2 * P, NR], F16, name="Fstage")
        with tc.tile_pool(name="fbounce", bufs=2) as fb:
            for half in range(2):
                t = fb.tile([P, NR], F16, tag="fb", name="fb")
                nc.sync.dma_start(out=t[:], in_=Fb_d[half * P:(half + 1) * P, :])
                nc.sync.dma_start(out=Fstage[half * P:(half + 1) * P, :], in_=t[:])
        nc.gpsimd.collective_compute(
            "AllGather", mybir.AluOpType.bypass, replica_groups=GROUPS,
            ins=[Fstage[:].opt()], outs=[FG[:].opt()])

        # ---------------- Phase A: project (token-sharded) ----------------
        hT_pool = ctx.enter_context(tc.tile_pool(name="hTp", bufs=3))
        with tc.tile_pool(name="xF", bufs=72) as xF, \
             tc.tile_pool(name="fwp", bufs=6) as fwp, \
             tc.tile_pool(name="tmpp", bufs=3) as tmpp, \
             tc.tile_pool(name="hp", bufs=12) as hp, \
             tc.tile_pool(name="psA", bufs=6, space="PSUM") as psA, \
             tc.tile_pool(name="psH", bufs=2, space="PSUM") as psH:
            xT_sb = []
            for dc in range(8):
                t = xF.tile([P, TOK], F16, tag="xT", name="xT")
                nc.sync.dma_start(out=t[:], in_=xT_d[dc * P:(dc + 1) * P, :])
                xT_sb.append(t)
            fw_sb = []
            for ti in range(3):
                t = fwp.tile([P, 4 * N], F16, tag="fw", name="fw")
                nc.sync.dma_start(
                    out=t[:].rearrange("p (u n) -> p u n", u=4),
                    in_=fw_d[ti][:].rearrange("(u p) n -> p u n", p=P))
                t32 = fwp.tile([P, 4 * N], F32, tag="fw32", name="fw32")
                nc.vector.tensor_copy(t32[:], t[:])
                fw_sb.append(t32)
            F_sb = {}  # (bank, dc, ns) -> [P, 512]
            for bank in range(2):
                for dc in range(8):
                    for ns in range(4):
                        t = xF.tile([P, 512], F16, tag="F", name="F")
                        r0 = dc * 2 * P + bank * P
                        nc.sync.dma_start(
                            out=t[:],
                            in_=FG[r0:r0 + P, ns * 512:(ns + 1) * 512])
                        F_sb[(bank, dc, ns)] = t

            h_sb = {}  # (ti, u) -> [P, R] f32
            for u in range(4):
                for bank, tensors in ((0, (0, 1)), (1, (2,))):
                    ps = []
                    for ns in range(4):
                        p = psA.tile([P, 512], F32, name="psA")
                        for dc in range(8):
                            nc.tensor.matmul(
                                p[:], xT_sb[dc][:, u * P:(u + 1) * P],
                                F_sb[(bank, dc, ns)][:],
                                start=(dc == 0), stop=(dc == 7))
                        ps.append(p)
                    for ti in tensors:
                        tmp = tmpp.tile([P, NR], F32, tag="tmp", name="tmp")
                        for ns in range(4):
                            p3 = ps[ns][:].rearrange("p (n r) -> p n r", n=8)
                            w3 = fw_sb[ti][:, u * N:(u + 1) * N] \
                                [:, ns * 8:(ns + 1) * 8] \
                                .unsqueeze(2).broadcast_to([P, 8, R])
                            tv = tmp[:].rearrange("p (r n) -> p n r", r=R)[
                                :, ns * 8:(ns + 1) * 8, :]
                            nc.vector.tensor_tensor(out=tv, in0=p3, in1=w3, op=MULT)
                        h = hp.tile([P, R], F32, tag="h", name="h")
                        nc.vector.reduce_sum(
                            out=h[:],
                            in_=tmp[:].rearrange("p (r n) -> p r n", r=R),
                            axis=AXX)
                        h_sb[(ti, u)] = h

            for ti in range(3):
                hT = hT_pool.tile([R, TOK], F16, tag="hT", name="hT")
                for u in range(4):
                    tp = psH.tile([R, P], F32, name="psH")
                    nc.tensor.transpose(tp[:], h_sb[(ti, u)][:], ident32[:])
                    cp(hT[:, u * P:(u + 1) * P], tp[:])
                nc.sync.dma_start(out=hT_stack[ti * R:(ti + 1) * R, :], in_=hT[:])

        nc.gpsimd.collective_compute(
            "AllGather", mybir.AluOpType.bypass, replica_groups=GROUPS,
            ins=[hT_stack[:].opt()], outs=[hT_gath[:].opt()])

        # h2[ti]: [128, T] f16; rows 0-63 and 64-127 both = full hT for ti
        h2pool = ctx.enter_context(tc.tile_pool(name="h2", bufs=3))
        h2 = []
        gv = hT_gath[:].rearrange("(c x) t -> c x t", c=8)
        for ti in range(3):
            t = h2pool.tile([P, T], F16, name="h2")
            src = gv[:, ti * R:(ti + 1) * R, :].rearrange("c r t -> r c t")
            for half in range(2):
                nc.sync.dma_start(
                    out=t[half * R:(half + 1) * R, :]
                        .rearrange("r (c t) -> r c t", c=8),
                    in_=src)
            h2.append(t)

        # ---------------- Phase B: restore (nr-sharded) ----------------
        with tc.tile_pool(name="Rp", bufs=6) as Rp, \
             tc.tile_pool(name="wrp", bufs=2) as wrp, \
             tc.tile_pool(name="gp", bufs=4) as gp, \
             tc.tile_pool(name="ysb", bufs=4) as ysb, \
             tc.tile_pool(name="psB", bufs=4, space="PSUM") as psB:
            R_sb = {}  # (bank, ch) -> [P, D] f16
            for bank, R_d in ((0, Rqk_d), (1, Rv_d)):
                for ch in range(2):
                    t = Rp.tile([P, D], F16, tag="R", name="R")
                    nc.sync.dma_start(out=t[:], in_=R_d[ch * P:(ch + 1) * P, :])
                    R_sb[(bank, ch)] = t
            for ti, bank in ((0, 0), (1, 0), (2, 1)):
                g = []
                for ch in range(2):
                    wr = wrp.tile([P, T], F16, tag="wr", name="wr")
                    for hh in range(2):
                        nn = 2 * ch + hh
                        nc.sync.dma_start(
                            out=wr[hh * R:(hh + 1) * R, :],
                            in_=rwT_d[ti][nn:nn + 1, :].broadcast_to([R, T]))
                    gt = gp.tile([P, T], F16, tag="g", name="g")
                    nc.vector.tensor_tensor(out=gt[:], in0=h2[ti][:], in1=wr[:],
                                            op=MULT)
                    g.append(gt)
                for tcn in range(8):
                    for dc in range(8):
                        ps = psB.tile([P, 512], F32, name="psB")
                        for ch in range(2):
                            nc.tensor.matmul(
                                ps[:],
                                R_sb[(bank, ch)][:, dc * P:(dc + 1) * P],
                                g[ch][:, tcn * 512:(tcn + 1) * 512],
                                start=(ch == 0), stop=(ch == 1))
                        y = ysb.tile([P, 512], F32, tag="y", name="y")
                        cp(y[:], ps[:])
                        nc.sync.dma_start(
                            out=yt_part[ti][dc * P:(dc + 1) * P,
                                            tcn * 512:(tcn + 1) * 512],
                            in_=y[:])

        for ti in range(3):
            nc.gpsimd.collective_compute(
                "ReduceScatter", ADD, replica_groups=GROUPS,
                ins=[yt_part[ti][:].opt()], outs=[yt_full[ti][:].opt()])

        # ---------------- Phase C: attention (2 heads x 2 batches) ----------
        qkv_pool = ctx.enter_context(tc.tile_pool(name="qkv", bufs=2))
        QT = qkv_pool.tile([P, T], F32, tag="QT", name="QT", bufs=1)
        KT = qkv_pool.tile([P, T], F32, tag="KT", name="KT", bufs=1)
        nc.sync.dma_start(out=QT[:], in_=yt_full[0][:])
        nc.sync.dma_start(out=KT[:], in_=yt_full[1][:])
        V_sb = []
        vsb = ctx.enter_context(tc.tile_pool(name="vsb", bufs=32))
        with tc.tile_pool(name="vload", bufs=1) as vload, \
             tc.tile_pool(name="psV", bufs=2, space="PSUM") as psV:
            Vt = vload.tile([P, T], F32, name="Vt")
            nc.sync.dma_start(out=Vt[:], in_=yt_full[2][:])
            for tt in range(32):
                tp = psV.tile([P, P], F32, name="psV")
                nc.tensor.transpose(tp[:], Vt[:, tt * P:(tt + 1) * P], ident32[:])
                v = vsb.tile([P, DL], F16, tag="V", name="V")
                cp(v[:], tp[:])
                V_sb.append(v)

        wot_pool = ctx.enter_context(tc.tile_pool(name="wot", bufs=1))
        WOT_sb = wot_pool.tile([P, D], F16, name="wot")
        nc.sync.dma_start(out=WOT_sb[:], in_=WOT_d[:])

        with tc.tile_pool(name="expS", bufs=2) as Ep, \
             tc.tile_pool(name="expT", bufs=4) as Tp, \
             tc.tile_pool(name="aop", bufs=4) as Ap, \
             tc.tile_pool(name="osb", bufs=4) as Op, \
             tc.tile_pool(name="small", bufs=32) as smp, \
             tc.tile_pool(name="psS", bufs=4, space="PSUM") as psS, \
             tc.tile_pool(name="psT", bufs=2, space="PSUM") as psT, \
             tc.tile_pool(name="psAV", bufs=1, space="PSUM") as psAV, \
             tc.tile_pool(name="psWO", bufs=1, space="PSUM") as psWO:
            for b in range(2):
                for qt in range(16):
                    L = (qt + 1) * P
                    nb = (L + 511) // 512
                    q0 = b * S + qt * P
                    ao_pair = Ap.tile([P, DL], F16, tag="ao", name="ao")
                    for head in range(2):
                        qoff = head * DH
                        ps_s = []
                        mxs = []
                        for kb in range(nb):
                            Ls = min(512, L - kb * 512)
                            p = psS.tile([P, 512], F32, name="psS")
                            nc.tensor.matmul(
                                p[:, :Ls],
                                _r(QT[qoff:qoff + DH, q0:q0 + P]),
                                _r(KT[qoff:qoff + DH,
                                      b * S + kb * 512:b * S + kb * 512 + Ls]),
                                start=True, stop=True)
                            if kb == nb - 1:
                                nc.vector.scalar_tensor_tensor(
                                    out=p[:, Ls - P:Ls], in0=maskU[:],
                                    scalar=-1e30, in1=p[:, Ls - P:Ls],
                                    op0=MULT, op1=ADD)
                            mx = smp.tile([P, 1], F32, tag="mx", name="mx")
                            nc.vector.reduce_max(out=mx[:], in_=p[:, :Ls],
                                                 axis=AXX)
                            ps_s.append(p)
                            mxs.append(mx)
                        m = mxs[0]
                        for mx in mxs[1:]:
                            m2 = smp.tile([P, 1], F32, tag="mx", name="mx")
                            nc.vector.tensor_max(m2[:], m[:], mx[:])
                            m = m2
                        negm = smp.tile([P, 1], F32, tag="mx", name="mx")
                        nc.vector.tensor_scalar_mul(negm[:], m[:], -0.125)
                        expS = Ep.tile([P, S], F16, tag="e", name="e")
                        dens = []
                        for kb in range(nb):
                            Ls = min(512, L - kb * 512)
                            den = smp.tile([P, 1], F32, tag="mx", name="mx")
                            nc.scalar.activation(
                                expS[:, kb * 512:kb * 512 + Ls],
                                ps_s[kb][:, :Ls], EXP,
                                bias=negm[:], scale=0.125, accum_out=den[:])
                            dens.append(den)
                        dtot = dens[0]
                        for den in dens[1:]:
                            d2 = smp.tile([P, 1], F32, tag="mx", name="mx")
                            nc.vector.tensor_tensor(out=d2[:], in0=dtot[:],
                                                    in1=den[:], op=ADD)
                            dtot = d2
                        recip = smp.tile([P, 1], F32, tag="mx", name="mx")
                        nc.vector.reciprocal(recip[:], dtot[:])
                        att = psAV.tile([P, DH], F32, name="psAV")
                        for tb in range(qt + 1):
                            tp = psT.tile([P, P], F16, name="psT")
                            nc.tensor.transpose(
                                tp[:], expS[:, tb * P:(tb + 1) * P], ident16[:])
                            eT = Tp.tile([P, P], F16, tag="eT", name="eT")
                            cp(eT[:], tp[:])
                            nc.tensor.matmul(
                                att[:], eT[:],
                                V_sb[b * 16 + tb][:, qoff:qoff + DH],
                                start=(tb == 0), stop=(tb == qt))
                        nc.vector.tensor_scalar_mul(
                            ao_pair[:, qoff:qoff + DH], att[:], recip[:])
                    # W_O for this (b, qt) block
                    tp = psT.tile([P, P], F16, name="psT")
                    nc.tensor.transpose(tp[:], ao_pair[:], ident16[:])
                    aoT = Ap.tile([P, P], F16, tag="aoT", name="aoT")
                    cp(aoT[:], tp[:])
                    for dh in range(2):
                        ps = psWO.tile([P, 512], F32, name="psWO")
                        nc.tensor.matmul(
                            ps[:], aoT[:], WOT_sb[:, dh * 512:(dh + 1) * 512],
                            start=True, stop=True)
                        osb = Op.tile([P, 512], F32, tag="osb", name="osb")
                        cp(osb[:], ps[:])
                        nc.sync.dma_start(
                            out=out_part[q0:q0 + P, dh * 512:(dh + 1) * 512],
                            in_=osb[:])

        nc.gpsimd.collective_compute(
            "ReduceScatter", ADD, replica_groups=GROUPS,
            ins=[out_part[:].opt()], outs=[out_rs[:].opt()])

        # downcast the reduced output slice to fp16 and store
        with tc.tile_pool(name="fin", bufs=4) as fin:
            for tt in range(4):
                f = fin.tile([P, D], F32, tag="f32", name="f32")
                nc.sync.dma_start(out=f[:], in_=out_rs[tt * P:(tt + 1) * P, :])
                f16t = fin.tile([P, D], F16, tag="f16", name="f16")
                nc.vector.tensor_copy(f16t[:], f[:])
                nc.sync.dma_start(out=out_d[tt * P:(tt + 1) * P, :], in_=f16t[:])
    nc.finalize()
    return nc


def kernel(x, fqk_weights_Q, fqk_weights_K, fv_weights,
           rqk_weights_Q, rqk_weights_K, rv_weights,
           f_qk, f_v, r_qk, r_v, W_O):
    f16 = np.float16
    x2 = np.asarray(x, np.float32).reshape(T, D)
    F_qk = np.asarray(f_qk, np.float32).transpose(1, 0, 2).reshape(D, NR)
    F_v = np.asarray(f_v, np.float32).transpose(1, 0, 2).reshape(D, NR)
    R_qk = np.asarray(r_qk, np.float32).reshape(NR, D).astype(f16)
    R_v = np.asarray(r_v, np.float32).reshape(NR, D).astype(f16)
    W_OT = np.asarray(W_O, np.float32).T.astype(f16)

    fw = [np.asarray(a, np.float32).reshape(T, N).astype(f16) for a in
          (fqk_weights_Q, fqk_weights_K, fv_weights)]
    rwT = [np.ascontiguousarray(
               np.asarray(a, np.float32).reshape(T, N).T).astype(f16)
           for a in (rqk_weights_Q, rqk_weights_K, rv_weights)]

    in_maps = []
    for c in range(8):
        m = {
            "xT": np.ascontiguousarray(x2[c * TOK:(c + 1) * TOK, :].T).astype(f16),
            "Fboth": np.ascontiguousarray(np.concatenate(
                [F_qk[c * P:(c + 1) * P, :], F_v[c * P:(c + 1) * P, :]],
                axis=0)).astype(f16),
            "Rqk": np.ascontiguousarray(R_qk[c * KRL:(c + 1) * KRL, :]),
            "Rv": np.ascontiguousarray(R_v[c * KRL:(c + 1) * KRL, :]),
            "WOT": np.ascontiguousarray(W_OT[c * DL:(c + 1) * DL, :]),
        }
        for name, arr in zip(("fwq", "fwk", "fwv"), fw):
            m[name] = np.ascontiguousarray(arr[c * TOK:(c + 1) * TOK, :])
        for name, arr in zip(("rwqT", "rwkT", "rwvT"), rwT):
            m[name] = np.ascontiguousarray(arr[c * NL:(c + 1) * NL, :])
        in_maps.append(m)

    if "nc" not in _CACHED:
        _CACHED["nc"] = build()
    res = run_bass_kernel_spmd(_CACHED["nc"], in_maps, list(range(8)))
    out = np.empty((T, D), np.float32)
    for c in range(8):
        out[c * TOK:(c + 1) * TOK, :] = res.results[c]["outp"].astype(np.float32)
    return out.reshape(B, S, D)


if __name__ == "__main__":
    d = np.load("/tmp/inputs.npz")
    out = kernel(**{k: d[k] for k in d.files})
    ref = np.load("/tmp/ref_out.npy")
    rel = np.linalg.norm(out - ref) / np.linalg.norm(ref)
    print("rel fro err:", rel)


# revision 9
# speedup vs baseline: 7.8067x; 1.6173x over previous
"""Trainium2 Bass kernel for nn_AttentionCircuit (mixture-routed attention).

Wire-transfer-minimal SPMD design (8 cores, single program). The axon tunnel
(~35 MB/s) dominates wall-clock, so every staged tensor is fp16 and sharded
with no cross-core replication of large data:

  - project: token-sharded (core c: global tokens [512c, 512c+512)); the
    F neuron banks are staged D-sharded (128 rows/core) and AllGathered
    on-chip; h vectors AllGathered (tiny).
  - restore: nr-sharded (core c: neurons [4c, 4c+4) = 256 of 2048 nr rows);
    partial Y^T accumulated in f32 and ReduceScattered over the D axis so
    each core ends with its 128-dim head slice (2 heads) for all tokens.
  - attention + W_O: head-sharded (2 heads x 2 batches per core); W_O
    partials ReduceScattered over tokens on-chip; each core returns a
    [512, 1024] fp16 slice of the final output.

On-chip numerics: fp16 matmuls with f32 PSUM accumulation everywhere except
the attention score matmul, which runs fp32r on f32 Q/K (softmax scores here
reach |s|~1900, so Q/K are never rounded below f32 after the restore).
"""
import sys
sys.path.insert(0, "/opt/trn_rl_repo")
import numpy as np
from contextlib import ExitStack

import concourse.bacc as bacc
import concourse.mybir as mybir
from concourse import tile
from concourse.masks import make_identity, make_upper_triangular
from concourse import bass2jax as _b2j
from concourse.bass_utils import (run_bass_kernel_spmd as _stock_run_spmd,
                                  BassKernelResults as _BKR)
import jax
from jax.experimental.shard_map import shard_map
from jax.sharding import Mesh, PartitionSpec, NamedSharding

B, S, D, R, H, DH, N = 2, 2048, 1024, 64, 16, 64, 32
NR = N * R            # 2048
T = B * S             # 4096 global tokens
P = 128
TOK = 512             # tokens per core (project shard)
NL = 4                # neurons per core (restore shard)
KRL = NL * R          # 256 local nr rows
DL = 128              # local d slice (2 heads) for attention/W_O
GROUPS = [[0, 1, 2, 3, 4, 5, 6, 7]]
F32 = mybir.dt.float32
F16 = mybir.dt.float16
F32R = mybir.dt.float32r
MULT = mybir.AluOpType.mult
ADD = mybir.AluOpType.add
AXX = mybir.AxisListType.X
EXP = mybir.ActivationFunctionType.Exp

_CACHED = {}
_EXEC_CACHE = {}


def _get_exec(nc, n_cores):
    """Build (once) the jitted SPMD executable for `nc`, mirroring the axon
    branch of concourse.bass_utils.run_bass_kernel_spmd, with two host-path
    savings: the jit closure is cached across calls, and the zero-filled
    ExternalOutput staging buffers live on-device instead of being shipped
    over the tunnel on every call. Inputs still transfer fully per call."""
    key = (id(nc), n_cores)
    if key in _EXEC_CACHE:
        return _EXEC_CACHE[key]
    _b2j.install_neuronx_cc_hook()
    partition_name = (nc.partition_id_tensor.name
                      if nc.partition_id_tensor else None)
    in_names, out_names, out_avals, zero_outs = [], [], [], []
    for alloc in nc.m.functions[0].allocations:
        if not isinstance(alloc, mybir.MemoryLocationSet):
            continue
        name = alloc.memorylocations[0].name
        if alloc.kind == "ExternalInput":
            if name != partition_name:
                in_names.append(name)
        elif alloc.kind == "ExternalOutput":
            shape = tuple(alloc.tensor_shape)
            dtype = mybir.dt.np(alloc.dtype)
            out_names.append(name)
            out_avals.append(jax.core.ShapedArray(shape, dtype))
            zero_outs.append(np.zeros(shape, dtype))
    n_params = len(in_names)
    bind_names = list(in_names) + list(out_names)
    if partition_name is not None:
        bind_names.append(partition_name)

    def _body(*args):
        operands = list(args)
        if partition_name is not None:
            operands.append(_b2j.partition_id_tensor())
        outs = _b2j._bass_exec_p.bind(
            *operands,
            out_avals=tuple(out_avals),
            in_names=tuple(bind_names),
            out_names=tuple(out_names),
            lowering_input_output_aliases=(),
            sim_require_finite=True,
            sim_require_nnan=True,
            nc=nc,
        )
        return tuple(outs)

    devices = jax.devices()[:n_cores]
    mesh = Mesh(np.asarray(devices), ("core",))
    n_outs = len(out_names)
    fn = jax.jit(
        shard_map(_body, mesh=mesh,
                  in_specs=(PartitionSpec("core"),) * (n_params + n_outs),
                  out_specs=(PartitionSpec("core"),) * n_outs,
                  check_rep=False),
        keep_unused=True)
    sh = NamedSharding(mesh, PartitionSpec("core"))
    zeros_dev = [jax.device_put(
                     np.zeros((n_cores * z.shape[0], *z.shape[1:]), z.dtype),
                     sh)
                 for z in zero_outs]
    info = (fn, in_names, out_names, out_avals, zeros_dev)
    _EXEC_CACHE[key] = info
    return info


def run_bass_kernel_spmd(nc, in_maps, core_ids, **kwargs):
    tr = kwargs.pop("trace", False)
    te = kwargs.pop("trace_events", False)
    if tr or te or kwargs:
        return _stock_run_spmd(nc, in_maps, core_ids, trace=tr,
                               trace_events=te, **kwargs)
    n = len(core_ids)
    fn, in_names, out_names, out_avals, zeros_dev = _get_exec(nc, n)
    concat_in = [
        np.concatenate([np.asarray(in_maps[c][k]) for c in range(n)], axis=0)
        for k in in_names
    ]
    out_arrs = fn(*concat_in, *zeros_dev)
    fetched = [np.asarray(a).reshape(n, *out_avals[i].shape)
               for i, a in enumerate(out_arrs)]
    results = [
        {name: fetched[i][c] for i, name in enumerate(out_names)}
        for c in range(n)
    ]
    return _BKR(results=results, instructions_and_trace=None,
                profile_json=None, exec_time_ns=None)


def _r(ap):
    return ap.bitcast(F32R)


def build():
    nc = bacc.Bacc(None, target_bir_lowering=False)

    def dp(name, shape, dt=F16, out=False):
        return nc.declare_dram_parameter(name, list(shape), dt, isOutput=out)

    xT_d = dp("xT", [D, TOK])                     # x[tok slice].T
    Fb_d = dp("Fboth", [2 * P, NR])               # [Fqk d-chunk; Fv d-chunk]
    fw_d = [dp(n, [TOK, N]) for n in ("fwq", "fwk", "fwv")]
    rwT_d = [dp(n, [NL, T]) for n in ("rwqT", "rwkT", "rwvT")]
    Rqk_d = dp("Rqk", [KRL, D])                   # nr-rows slice, all d
    Rv_d = dp("Rv", [KRL, D])
    WOT_d = dp("WOT", [DL, D])                    # W_O.T rows [128c:+128]
    out_d = dp("outp", [TOK, D], out=True)        # final rows [512c:+512)

    tog = [0]

    def cp(out, in_):
        tog[0] ^= 1
        if tog[0]:
            nc.scalar.copy(out, in_)
        else:
            nc.vector.tensor_copy(out, in_)

    with ExitStack() as ctx:
        tc = ctx.enter_context(tile.TileContext(nc))
        const = ctx.enter_context(tc.tile_pool(name="const", bufs=1))
        ident32 = const.tile([P, P], F32, name="id32")
        make_identity(nc, ident32[:])
        ident16 = const.tile([P, P], F16, name="id16")
        make_identity(nc, ident16[:])
        maskU = const.tile([P, P], F32, name="maskU")
        make_upper_triangular(nc, maskU[:], val=1.0, diag=False)

        dram = ctx.enter_context(tc.tile_pool(name="dram", bufs=1, space="DRAM"))
        FG = dram.tile([8 * 2 * P, NR], F16, name="FG")          # gathered F
        hT_stack = dram.tile([3 * R, TOK], F16, name="hTstack")
        hT_gath = dram.tile([8 * 3 * R, TOK], F16, name="hTgath")
        yt_part = [dram.tile([D, T], F32, name=f"ytp{i}") for i in range(3)]
        yt_full = [dram.tile([DL, T], F32, name=f"ytf{i}") for i in range(3)]
        out_part = dram.tile([T, D], F32, name="outpart")
        out_rs = dram.tile([TOK, D], F32, name="outrs")

        # -------- AllGather the D-sharded F banks (1MB in, 8MB out) --------
        # (collectives cannot read IO tensors directly: bounce via SBUF)
        Fstage = dram.tile([2 * P, NR], F16, name="Fstage")
        with tc.tile_pool(name="fbounce", bufs=2) as fb:
            for half in range(2):
                t = fb.tile([P, NR], F16, tag="fb", name="fb")
                nc.sync.dma_start(out=t[:], in_=Fb_d[half * P:(half + 1) * P, :])
                nc.sync.dma_start(out=Fstage[half * P:(half + 1) * P, :], in_=t[:])
        nc.gpsimd.collective_compute(
            "AllGather", mybir.AluOpType.bypass, replica_groups=GROUPS,
            ins=[Fstage[:].opt()], outs=[FG[:].opt()])

        # ---------------- Phase A: project (token-sharded) ----------------
        hT_pool = ctx.enter_context(tc.tile_pool(name="hTp", bufs=3))
        with tc.tile_pool(name="xF", bufs=72) as xF, \
             tc.tile_pool(name="fwp", bufs=6) as fwp, \
             tc.tile_pool(name="tmpp", bufs=3) as tmpp, \
             tc.tile_pool(name="hp", bufs=12) as hp, \
             tc.tile_pool(name="psA", bufs=6, space="PSUM") as psA, \
             tc.tile_pool(name="psH", bufs=2, space="PSUM") as psH:
            xT_sb = []
            for dc in range(8):
                t = xF.tile([P, TOK], F16, tag="xT", name="xT")
                nc.sync.dma_start(out=t[:], in_=xT_d[dc * P:(dc + 1) * P, :])
                xT_sb.append(t)
            fw_sb = []
            for ti in range(3):
                t = fwp.tile([P, 4 * N], F16, tag="fw", name="fw")
                nc.sync.dma_start(
                    out=t[:].rearrange("p (u n) -> p u n", u=4),
                    in_=fw_d[ti][:].rearrange("(u p) n -> p u n", p=P))
                t32 = fwp.tile([P, 4 * N], F32, tag="fw32", name="fw32")
                nc.vector.tensor_copy(t32[:], t[:])
                fw_sb.append(t32)
            F_sb = {}  # (bank, dc, ns) -> [P, 512]
            for bank in range(2):
                for dc in range(8):
                    for ns in range(4):
                        t = xF.tile([P, 512], F16, tag="F", name="F")
                        r0 = dc * 2 * P + bank * P
                        nc.sync.dma_start(
                            out=t[:],
                            in_=FG[r0:r0 + P, ns * 512:(ns + 1) * 512])
                        F_sb[(bank, dc, ns)] = t

            h_sb = {}  # (ti, u) -> [P, R] f32
            for u in range(4):
                for bank, tensors in ((0, (0, 1)), (1, (2,))):
                    ps = []
                    for ns in range(4):
                        p = psA.tile([P, 512], F32, name="psA")
                        for dc in range(8):
                            nc.tensor.matmul(
                                p[:], xT_sb[dc][:, u * P:(u + 1) * P],
                                F_sb[(bank, dc, ns)][:],
                                start=(dc == 0), stop=(dc == 7))
                        ps.append(p)
                    for ti in tensors:
                        tmp = tmpp.tile([P, NR], F32, tag="tmp", name="tmp")
                        for ns in range(4):
                            p3 = ps[ns][:].rearrange("p (n r) -> p n r", n=8)
                            w3 = fw_sb[ti][:, u * N:(u + 1) * N] \
                                [:, ns * 8:(ns + 1) * 8] \
                                .unsqueeze(2).broadcast_to([P, 8, R])
                            tv = tmp[:].rearrange("p (r n) -> p n r", r=R)[
                                :, ns * 8:(ns + 1) * 8, :]
                            nc.vector.tensor_tensor(out=tv, in0=p3, in1=w3, op=MULT)
                        h = hp.tile([P, R], F32, tag="h", name="h")
                        nc.vector.reduce_sum(
                            out=h[:],
                            in_=tmp[:].rearrange("p (r n) -> p r n", r=R),
                            axis=AXX)
                        h_sb[(ti, u)] = h

            for ti in range(3):
                hT = hT_pool.tile([R, TOK], F16, tag="hT", name="hT")
                for u in range(4):
                    tp = psH.tile([R, P], F32, name="psH")
                    nc.tensor.transpose(tp[:], h_sb[(ti, u)][:], ident32[:])
                    cp(hT[:, u * P:(u + 1) * P], tp[:])
                nc.sync.dma_start(out=hT_stack[ti * R:(ti + 1) * R, :], in_=hT[:])

        nc.gpsimd.collective_compute(
            "AllGather", mybir.AluOpType.bypass, replica_groups=GROUPS,
            ins=[hT_stack[:].opt()], outs=[hT_gath[:].opt()])

        # h2[ti]: [128, T] f16; rows 0-63 and 64-127 both = full hT for ti
        h2pool = ctx.enter_context(tc.tile_pool(name="h2", bufs=3))
        h2 = []
        gv = hT_gath[:].rearrange("(c x) t -> c x t", c=8)
        for ti in range(3):
            t = h2pool.tile([P, T], F16, name="h2")
            src = gv[:, ti * R:(ti + 1) * R, :].rearrange("c r t -> r c t")
            for half in range(2):
                nc.sync.dma_start(
                    out=t[half * R:(half + 1) * R, :]
                        .rearrange("r (c t) -> r c t", c=8),
                    in_=src)
            h2.append(t)

        # ---------------- Phase B: restore (nr-sharded) ----------------
        with tc.tile_pool(name="Rp", bufs=6) as Rp, \
             tc.tile_pool(name="wrp", bufs=2) as wrp, \
             tc.tile_pool(name="gp", bufs=4) as gp, \
             tc.tile_pool(name="ysb", bufs=4) as ysb, \
             tc.tile_pool(name="psB", bufs=4, space="PSUM") as psB:
            R_sb = {}  # (bank, ch) -> [P, D] f16
            for bank, R_d in ((0, Rqk_d), (1, Rv_d)):
                for ch in range(2):
                    t = Rp.tile([P, D], F16, tag="R", name="R")
                    nc.sync.dma_start(out=t[:], in_=R_d[ch * P:(ch + 1) * P, :])
                    R_sb[(bank, ch)] = t
            for ti, bank in ((0, 0), (1, 0), (2, 1)):
                g = []
                for ch in range(2):
                    wr = wrp.tile([P, T], F16, tag="wr", name="wr")
                    for hh in range(2):
                        nn = 2 * ch + hh
                        nc.sync.dma_start(
                            out=wr[hh * R:(hh + 1) * R, :],
                            in_=rwT_d[ti][nn:nn + 1, :].broadcast_to([R, T]))
                    gt = gp.tile([P, T], F16, tag="g", name="g")
                    nc.vector.tensor_tensor(out=gt[:], in0=h2[ti][:], in1=wr[:],
                                            op=MULT)
                    g.append(gt)
                for tcn in range(8):
                    for dc in range(8):
                        ps = psB.tile([P, 512], F32, name="psB")
                        for ch in range(2):
                            nc.tensor.matmul(
                                ps[:],
                                R_sb[(bank, ch)][:, dc * P:(dc + 1) * P],
                                g[ch][:, tcn * 512:(tcn + 1) * 512],
                                start=(ch == 0), stop=(ch == 1))
                        y = ysb.tile([P, 512], F32, tag="y", name="y")
                        cp(y[:], ps[:])
                        nc.sync.dma_start(
                            out=yt_part[ti][dc * P:(dc + 1) * P,
                                            tcn * 512:(tcn + 1) * 512],
                            in_=y[:])

        for ti in range(3):
            nc.gpsimd.collective_compute(
                "ReduceScatter", ADD, replica_groups=GROUPS,
                ins=[yt_part[ti][:].opt()], outs=[yt_full[ti][:].opt()])

        # ---------------- Phase C: attention (2 heads x 2 batches) ----------
        qkv_pool = ctx.enter_context(tc.tile_pool(name="qkv", bufs=2))
        QT = qkv_pool.tile([P, T], F32, tag="QT", name="QT", bufs=1)
        KT = qkv_pool.tile([P, T], F32, tag="KT", name="KT", bufs=1)
        nc.sync.dma_start(out=QT[:], in_=yt_full[0][:])
        nc.sync.dma_start(out=KT[:], in_=yt_full[1][:])
        V_sb = []
        vsb = ctx.enter_context(tc.tile_pool(name="vsb", bufs=32))
        with tc.tile_pool(name="vload", bufs=1) as vload, \
             tc.tile_pool(name="psV", bufs=2, space="PSUM") as psV:
            Vt = vload.tile([P, T], F32, name="Vt")
            nc.sync.dma_start(out=Vt[:], in_=yt_full[2][:])
            for tt in range(32):
                tp = psV.tile([P, P], F32, name="psV")
                nc.tensor.transpose(tp[:], Vt[:, tt * P:(tt + 1) * P], ident32[:])
                v = vsb.tile([P, DL], F16, tag="V", name="V")
                cp(v[:], tp[:])
                V_sb.append(v)

        wot_pool = ctx.enter_context(tc.tile_pool(name="wot", bufs=1))
        WOT_sb = wot_pool.tile([P, D], F16, name="wot")
        nc.sync.dma_start(out=WOT_sb[:], in_=WOT_d[:])

        with tc.tile_pool(name="expS", bufs=2) as Ep, \
             tc.tile_pool(name="expT", bufs=4) as Tp, \
             tc.tile_pool(name="aop", bufs=4) as Ap, \
             tc.tile_pool(name="osb", bufs=4) as Op, \
             tc.tile_pool(name="small", bufs=32) as smp, \
             tc.tile_pool(name="psS", bufs=4, space="PSUM") as psS, \
             tc.tile_pool(name="psT", bufs=2, space="PSUM") as psT, \
             tc.tile_pool(name="psAV", bufs=1, space="PSUM") as psAV, \
             tc.tile_pool(name="psWO", bufs=1, space="PSUM") as psWO:
            for b in range(2):
                for qt in range(16):
                    L = (qt + 1) * P
                    nb = (L + 511) // 512
                    q0 = b * S + qt * P
                    ao_pair = Ap.tile([P, DL], F16, tag="ao", name="ao")
                    for head in range(2):
                        qoff = head * DH
                        ps_s = []
                        mxs = []
                        for kb in range(nb):
                            Ls = min(512, L - kb * 512)
                            p = psS.tile([P, 512], F32, name="psS")
                            nc.tensor.matmul(
                                p[:, :Ls],
                                _r(QT[qoff:qoff + DH, q0:q0 + P]),
                                _r(KT[qoff:qoff + DH,
                                      b * S + kb * 512:b * S + kb * 512 + Ls]),
                                start=True, stop=True)
                            if kb == nb - 1:
                                nc.vector.scalar_tensor_tensor(
                                    out=p[:, Ls - P:Ls], in0=maskU[:],
                                    scalar=-1e30, in1=p[:, Ls - P:Ls],
                                    op0=MULT, op1=ADD)
                            mx = smp.tile([P, 1], F32, tag="mx", name="mx")
                            nc.vector.reduce_max(out=mx[:], in_=p[:, :Ls],
                                                 axis=AXX)
                            ps_s.append(p)
                            mxs.append(mx)
                        m = mxs[0]
                        for mx in mxs[1:]:
                            m2 = smp.tile([P, 1], F32, tag="mx", name="mx")
                            nc.vector.tensor_max(m2[:], m[:], mx[:])
                            m = m2
                        negm = smp.tile([P, 1], F32, tag="mx", name="mx")
                        nc.vector.tensor_scalar_mul(negm[:], m[:], -0.125)
                        expS = Ep.tile([P, S], F16, tag="e", name="e")
                        dens = []
                        for kb in range(nb):
                            Ls = min(512, L - kb * 512)
                            den = smp.tile([P, 1], F32, tag="mx", name="mx")
                            nc.scalar.activation(
                                expS[:, kb * 512:kb * 512 + Ls],
                                ps_s[kb][:, :Ls], EXP,
                                bias=negm[:], scale=0.125, accum_out=den[:])
                            dens.append(den)
                        dtot = dens[0]
                        for den in dens[1:]:
                            d2 = smp.tile([P, 1], F32, tag="mx", name="mx")
                            nc.vector.tensor_tensor(out=d2[:], in0=dtot[:],
                                                    in1=den[:], op=ADD)
                            dtot = d2
                        recip = smp.tile([P, 1], F32, tag="mx", name="mx")
                        nc.vector.reciprocal(recip[:], dtot[:])
                        att = psAV.tile([P, DH], F32, name="psAV")
                        for tb in range(qt + 1):
                            tp = psT.tile([P, P], F16, name="psT")
                            nc.tensor.transpose(
                                tp[:], expS[:, tb * P:(tb + 1) * P], ident16[:])
                            eT = Tp.tile([P, P], F16, tag="eT", name="eT")
                            cp(eT[:], tp[:])
                            nc.tensor.matmul(
                                att[:], eT[:],
                                V_sb[b * 16 + tb][:, qoff:qoff + DH],
                                start=(tb == 0), stop=(tb == qt))
                        nc.vector.tensor_scalar_mul(
                            ao_pair[:, qoff:qoff + DH], att[:], recip[:])
                    # W_O for this (b, qt) block
                    tp = psT.tile([P, P], F16, name="psT")
                    nc.tensor.transpose(tp[:], ao_pair[:], ident16[:])
                    aoT = Ap.tile([P, P], F16, tag="aoT", name="aoT")
                    cp(aoT[:], tp[:])
                    for dh in range(2):
                        ps = psWO.tile([P, 512], F32, name="psWO")
                        nc.tensor.matmul(
                            ps[:], aoT[:], WOT_sb[:, dh * 512:(dh + 1) * 512],
                            start=True, stop=True)
                        osb = Op.tile([P, 512], F32, tag="osb", name="osb")
                        cp(osb[:], ps[:])
                        nc.sync.dma_start(
                            out=out_part[q0:q0 + P, dh * 512:(dh + 1) * 512],
                            in_=osb[:])

        nc.gpsimd.collective_compute(
            "ReduceScatter", ADD, replica_groups=GROUPS,
            ins=[out_part[:].opt()], outs=[out_rs[:].opt()])

        # downcast the reduced output slice to fp16 and store
        with tc.tile_pool(name="fin", bufs=4) as fin:
            for tt in range(4):
                f = fin.tile([P, D], F32, tag="f32", name="f32")
                nc.sync.dma_start(out=f[:], in_=out_rs[tt * P:(tt + 1) * P, :])
                f16t = fin.tile([P, D], F16, tag="f16", name="f16")
                nc.vector.tensor_copy(f16t[:], f[:])
                nc.sync.dma_start(out=out_d[tt * P:(tt + 1) * P, :], in_=f16t[:])
    nc.finalize()
    return nc


def kernel(x, fqk_weights_Q, fqk_weights_K, fv_weights,
           rqk_weights_Q, rqk_weights_K, rv_weights,
           f_qk, f_v, r_qk, r_v, W_O):
    f16 = np.float16
    x2 = np.asarray(x, np.float32).reshape(T, D)
    F_qk = np.asarray(f_qk, np.float32).transpose(1, 0, 2).reshape(D, NR)
    F_v = np.asarray(f_v, np.float32).transpose(1, 0, 2).reshape(D, NR)
    R_qk = np.asarray(r_qk, np.float32).reshape(NR, D).astype(f16)
    R_v = np.asarray(r_v, np.float32).reshape(NR, D).astype(f16)
    W_OT = np.asarray(W_O, np.float32).T.astype(f16)

    fw = [np.asarray(a, np.float32).reshape(T, N).astype(f16) for a in
          (fqk_weights_Q, fqk_weights_K, fv_weights)]
    rwT = [np.ascontiguousarray(
               np.asarray(a, np.float32).reshape(T, N).T).astype(f16)
           for a in (rqk_weights_Q, rqk_weights_K, rv_weights)]

    in_maps = []
    for c in range(8):
        m = {
            "xT": np.ascontiguousarray(x2[c * TOK:(c + 1) * TOK, :].T).astype(f16),
            "Fboth": np.ascontiguousarray(np.concatenate(
                [F_qk[c * P:(c + 1) * P, :], F_v[c * P:(c + 1) * P, :]],
                axis=0)).astype(f16),
            "Rqk": np.ascontiguousarray(R_qk[c * KRL:(c + 1) * KRL, :]),
            "Rv": np.ascontiguousarray(R_v[c * KRL:(c + 1) * KRL, :]),
            "WOT": np.ascontiguousarray(W_OT[c * DL:(c + 1) * DL, :]),
        }
        for name, arr in zip(("fwq", "fwk", "fwv"), fw):
            m[name] = np.ascontiguousarray(arr[c * TOK:(c + 1) * TOK, :])
        for name, arr in zip(("rwqT", "rwkT", "rwvT"), rwT):
            m[name] = np.ascontiguousarray(arr[c * NL:(c + 1) * NL, :])
        in_maps.append(m)

    if "nc" not in _CACHED:
        _CACHED["nc"] = build()
    res = run_bass_kernel_spmd(_CACHED["nc"], in_maps, list(range(8)))
    out = np.empty((T, D), np.float32)
    for c in range(8):
        out[c * TOK:(c + 1) * TOK, :] = res.results[c]["outp"].astype(np.float32)
    return out.reshape(B, S, D)


if __name__ == "__main__":
    d = np.load("/tmp/inputs.npz")
    out = kernel(**{k: d[k] for k in d.files})
    ref = np.load("/tmp/ref_out.npy")
    rel = np.linalg.norm(out - ref) / np.linalg.norm(ref)
    print("rel fro err:", rel)
